# revision 38
# baseline (speedup 1.0000x reference)
"""L2 contrastive loss (margin=1.0) on 8 Trainium2 NeuronCores.

loss = (sum_{i!=j} relu(1 - d_ij)^2 + sum_i d_ii^2) / (2N),
d_ij = ||f1_i - f2_j||.

Strategy (certificate kernel): each core verifies on-device that every
pair in its 1024 x 8192 block of the distance matrix satisfies
d_ij >= 1 (so every hinge term is exactly 0) and computes its share of
the exact diagonal sum_i ||f1_i - f2_i||^2.  Host combines:
loss = diag/(2N) when every core's screen is zero, else falls back to
an exact host computation.

Device algorithm per core:
  * PE: z_ij = f1q_i . (2 f2q)_j in fp8e4m3 with DoubleRow perf mode
    (K=128 as [64,2] k-tiles, 0.5 cycles/col) into a [128, 4096] PSUM
    ring of 8 banks (16 wraps of 8 banks cover the 8 i-tiles x 16
    j-groups).  fp8 keeps the PE well ahead of the screens.
  * Screens drain the ring with both PSUM-capable engines concurrently
    (DVE tensor_scalar and ACT activation, each computing the sum of
    relu(z + bias) into per-op accumulator columns; the sum is zero iff
    every covered pair is certified outside the margin).  A DVE/ACT
    instruction may read only ONE input from PSUM, so 1 elem/lane/cycle
    per engine is the hard PSUM drain rate; op spans come from a
    host-side greedy planner that drifts around the ring so bank
    refills hide behind the opposite engine.
  * Certificate: relu(z + bias) == 0 iff z <= -bias; bias column =
    -(min over covered (tile, group) of [sq1_i + min sq2_g - 1 -
    margin]), margin a rigorous bound on the fp8 quantization error
    (|2 dot - z| <= n1*|dn2| + |dn1|*n2q, Cauchy-Schwarz with exactly-
    computed host-side error norms).  Rows are striped so partition p
    holds 8 consecutive norm-sorted rows across the 8 i-tiles, keeping
    thresholds tight when one op spans several tiles.
  * Diagonal: ACT Square+accum over host-computed bf16 (f1 - f2) rows.
Host: loss = sum(diag partials) / (2N) when every core's screen passes;
otherwise falls back to an exact full computation.
"""

import numpy as np
import ml_dtypes

N = 8192
D = 128
NCORES = 8
R = N // NCORES      # 1024 rows of feature1 per core
NT = 8               # i-tiles per core (128 rows each)
NG = 16              # j-groups (512 cols each)
NBANK = 8            # physical PSUM banks (512 fp32 each)
GBANKS = NT * NG     # 128 global bank-fills per core
BANKW = 512

TRACE = False       # test harness can set kernel.TRACE = True
TRACE_KWARGS = {}
LAST_RESULT = None  # BassKernelResults of the last run
LAST_SCREEN = None  # (diag_total, screen_total) of the last run

_BASS_CACHE = {}

# ---- screen planner ------------------------------------------------------
# Cost model (ns): DVE tensor_scalar op of W elems: W*1.0417 + 175;
# ACT relu op of L elems: L*0.8333 + 420.  PE refills a bank in ~120ns
# (fp8 DoubleRow) + ~250ns sem turnaround.
RING = NBANK * BANKW        # 4096 fp32 per partition
TOTEL = GBANKS * BANKW      # 65536 elems per partition per core


def _plan_once(d_el, a_el):
    """Greedy elem-granular ring schedule. Screens are flat spans of the
    global element stream (bank B holds elems [B*512, (B+1)*512)); spans
    never cross a ring-wrap boundary. Returns (ops, est_ns); each op is
    (engine, e0, el) with e0 global elem offset."""
    dve_c = 1.0417
    act_c = 0.8333
    dve_oh = 175.0
    act_oh = 420.0
    sem = 250.0
    fill = 120.0
    eng_free = {"d": 0.0, "a": 2800.0}
    # screen-completion time per physical bank (previous ring tenant)
    freed = [0.0] * NBANK
    filled = [0.0] * NBANK   # fill completion, current tenant
    pe_t = 2200.0            # PE free time (first rhs chunk landed)
    next_fill = 0            # next global bank PE will fill
    # prefill what we can (bounded by screens of previous tenants)
    ops = []
    E = 0                    # next global element to screen
    diag_done = False

    def fill_to(gbank):
        nonlocal pe_t, next_fill
        while next_fill <= gbank:
            b = next_fill % NBANK
            start = max(pe_t, freed[b] + sem if next_fill >= NBANK else pe_t)
            pe_t = start + fill
            filled[b] = pe_t
            next_fill += 1

    while E < TOTEL:
        e = "d" if eng_free["d"] <= eng_free["a"] else "a"
        tgt = d_el if e == "d" else a_el
        el = min(tgt, RING - (E % RING), TOTEL - E)
        b0, b1 = E // BANKW, (E + el - 1) // BANKW
        fill_to(b1)
        ready = max(filled[b % NBANK] for b in range(b0, b1 + 1))
        start = max(eng_free[e], ready + sem / 2)
        if e == "d":
            busy = el * dve_c + dve_oh
        else:
            busy = el * act_c + act_oh
        end = start + busy
        ops.append((e, E, el))
        eng_free[e] = end
        for b in range(b0, b1 + 1):
            freed[b % NBANK] = end
        E += el
        if not diag_done and e == "a" and eng_free["a"] > 5200.0:
            ops.append(("g", 0, R))
            eng_free["a"] += R * act_c + act_oh
            diag_done = True
    if not diag_done:
        ops.append(("g", 0, R))
        eng_free["a"] += R * act_c + act_oh
    return ops, max(eng_free.values())


PLAN_D_EL = 1664     # DVE op span (elems); tuned against measured traces
PLAN_A_EL = 1792     # ACT op span (elems)


def _plan():
    return _plan_once(PLAN_D_EL, PLAN_A_EL)


def _build_bass(ops):
    import concourse.bacc as bacc
    import concourse.mybir as mybir
    import concourse.tile as tile

    fp32 = mybir.dt.float32
    bf16 = mybir.dt.bfloat16
    fp8 = mybir.dt.float8e4
    Alu = mybir.AluOpType
    Act = mybir.ActivationFunctionType
    DR = mybir.MatmulPerfMode.DoubleRow

    nd = sum(1 for e, _, _ in ops if e == "d")
    na = sum(1 for e, _, _ in ops if e == "a")
    max_d = max((el for e, _, el in ops if e == "d"), default=2)
    max_a = max((el for e, _, el in ops if e == "a"), default=2)

    nc = bacc.Bacc("TRN2", target_bir_lowering=False, debug=False,
                   num_devices=NCORES)

    # ---- DRAM I/O ----
    # fp8 of (2*f2_sorted), group-major k-tiled: [64, 2g+i, j']
    d_f2r = nc.dram_tensor("f2r", [64, NG * 2, BANKW], fp8, kind="ExternalInput")
    # fp8 of f1 rows, k-tiled per i-tile: [64, i, t*128+m]
    d_f1l = nc.dram_tensor("f1l", [64, 2, NT * 128], fp8, kind="ExternalInput")
    # per-op screen bias columns (DVE / ACT)
    d_biad = nc.dram_tensor("biad", [128, max(nd, 1)], fp32, kind="ExternalInput")
    d_bia = nc.dram_tensor("bia", [128, max(na, 1)], fp32, kind="ExternalInput")
    # bf16 host-computed (f1 - f2) rows for the exact diagonal
    d_diff = nc.dram_tensor("diff", [128, R], bf16, kind="ExternalInput")
    # out[0,0] = sum_i ||f1_i - f2_i||^2 ; out[1,0] = screen (0 iff no hinge)
    d_out = nc.dram_tensor("out", [2, 1], fp32, kind="ExternalOutput")

    with tile.TileContext(nc) as tc:
        with (
            tc.tile_pool(name="singles", bufs=1) as singles,
            tc.tile_pool(name="chunks", bufs=1) as chunks,
        ):
            # ---- input DMAs (sync HWDGE ring is FIFO: order matters) ----
            s_f2r = chunks.tile([64, NG * 2, BANKW], fp8, tag="f2r")
            # the first DVE op's groups land first
            nc.sync.dma_start(s_f2r[:, 0:8, :], d_f2r[:, 0:8, :])
            s_f1l = singles.tile([64, 2, NT * 128], fp8, tag="f1l")
            nc.sync.dma_start(s_f1l[:, :, :], d_f1l[:, :, :])
            s_bia = singles.tile([128, max(na, 1)], fp32, tag="bia")
            nc.sync.dma_start(s_bia[:, :], d_bia[:, :])
            s_biad = singles.tile([128, max(nd, 1)], fp32, tag="biad")
            nc.sync.dma_start(s_biad[:, :], d_biad[:, :])
            nc.sync.dma_start(s_f2r[:, 8:20, :], d_f2r[:, 8:20, :])
            s_diff = singles.tile([128, R], bf16, tag="diff_in")
            nc.sync.dma_start(s_diff[:, :], d_diff[:, :])
            nc.sync.dma_start(s_f2r[:, 20:32, :], d_f2r[:, 20:32, :])

            # ---- accumulators & trash ----
            acc_diag = singles.tile([128, 1], fp32, tag="acc_diag")
            acc_d = singles.tile([128, max(nd, 1)], fp32, tag="acc_d")
            acc_a = singles.tile([128, max(na, 1)], fp32, tag="acc_a")
            trash_d = singles.tile([128, max_d], bf16, tag="trash_d")
            trash_a = singles.tile([128, max_a], bf16, tag="trash_a")
            trash32 = singles.tile([128, R], fp32, tag="trash32")
            m_final = singles.tile([128, 2], fp32, tag="m_final")
            ones_sb = singles.tile([128, 1], fp32, tag="ones_sb")
            red_d = singles.tile([128, 1], fp32, tag="red_d")
            red_a = singles.tile([128, 1], fp32, tag="red_a")
            out_sb = singles.tile([2, 1], fp32, tag="out_sb")

            nc.vector.memset(ones_sb[:, :], 1.0)

            # ACT warmup: trigger the Relu act-table load during the DMA
            # lead-in so the first real screen doesn't pay ~1.3us for it.
            nc.scalar.activation(
                m_final[:, 0:1], ones_sb[:, :], Act.Relu, scale=1.0
            )

            # ---- main loop: PE ring fill + planned screens, emitted
            # interleaved in plan order (the tile scheduler keeps per-
            # engine emission order, so screens must be emitted before
            # the fills that overwrite their banks) ----
            with tc.tile_pool(name="psum_main", bufs=1, space="PSUM") as pp:
                big = pp.tile([128, RING], fp32, tag="big")
                next_mm = 0

                def fill_banks_to(gbank):
                    nonlocal next_mm
                    while next_mm <= gbank:
                        B = next_mm
                        t, g = B // NG, B % NG
                        off = (B % NBANK) * BANKW
                        nc.tensor.matmul(
                            big[:, off: off + BANKW],
                            lhsT=s_f1l[:, :, t * 128: (t + 1) * 128],
                            rhs=s_f2r[:, 2 * g: 2 * g + 2, :],
                            start=True,
                            stop=True,
                            perf_mode=DR,
                        )
                        next_mm += 1

                i_d = 0
                i_a = 0
                for e, e0, el in ops:
                    if e != "g":
                        fill_banks_to((e0 + el - 1) // BANKW)
                    off = e0 % RING
                    if e == "g":
                        # exact diagonal: sum_i ||f1_i - f2_i||^2
                        nc.scalar.activation(
                            trash32[:, :],
                            s_diff[:, :],
                            Act.Square,
                            accum_out=acc_diag[:, 0:1],
                        )
                    elif e == "d":
                        nc.vector.tensor_scalar(
                            trash_d[:, 0:el],
                            big[:, off: off + el],
                            s_biad[:, i_d: i_d + 1],
                            0.0,
                            Alu.add,
                            Alu.max,
                            accum_out=acc_d[:, i_d: i_d + 1],
                        )
                        i_d += 1
                    else:
                        nc.scalar.activation(
                            trash_a[:, 0:el],
                            big[:, off: off + el],
                            Act.Relu,
                            bias=s_bia[:, i_a: i_a + 1],
                            scale=1.0,
                            accum_out=acc_a[:, i_a: i_a + 1],
                        )
                        i_a += 1

            # ---- final reduction ----
            nc.vector.tensor_reduce(
                red_d[:, :], acc_d[:, :], axis=mybir.AxisListType.X, op=Alu.add
            )
            nc.vector.tensor_reduce(
                red_a[:, :], acc_a[:, :], axis=mybir.AxisListType.X, op=Alu.add
            )
            nc.vector.tensor_copy(m_final[:, 0:1], acc_diag[:, 0:1])
            nc.vector.tensor_add(m_final[:, 1:2], red_d[:, :], red_a[:, :])

            with tc.tile_pool(name="psum_fin", bufs=1, space="PSUM") as pf_pool:
                pf = pf_pool.tile([2, 1], fp32, tag="pf")
                nc.tensor.matmul(
                    pf[:, :], lhsT=m_final[:, :], rhs=ones_sb[:, :],
                    start=True, stop=True,
                )
                nc.vector.tensor_copy(out_sb[:, :], pf[:, :])

            nc.sync.dma_start(d_out[:, :], out_sb[:, :])

    nc.compile()
    return nc


def _get_nc():
    if "nc" not in _BASS_CACHE:
        ops, _ = _plan()
        _BASS_CACHE["nc"] = (_build_bass(ops), ops)
    return _BASS_CACHE["nc"]


def _full_numpy_fallback(f1, f2):
    """Exact reference computation (only used if the screen certificate
    fails, i.e. some pair has d_ij close to or inside the margin)."""
    f1 = f1.astype(np.float32)
    f2 = f2.astype(np.float32)
    n = f1.shape[0]
    sq1 = np.sum(f1 * f1, axis=1)
    sq2 = np.sum(f2 * f2, axis=1)
    total = np.float64(0.0)
    chunk = 512
    for s in range(0, n, chunk):
        e = min(s + chunk, n)
        d2 = sq1[s:e, None] + sq2[None, :] - 2.0 * (f1[s:e] @ f2.T)
        d = np.sqrt(np.maximum(d2, 0.0))
        c = np.maximum(1.0 - d, 0.0)
        for r in range(s, e):
            c[r - s, r] = 0.0
        total += np.float64(np.sum(c * c))
    total += np.float64(np.sum((f1 - f2) ** 2))
    return np.float32(total / (2.0 * n))


def kernel(feature1, feature2):
    global LAST_RESULT, LAST_SCREEN
    from concourse.bass_utils import run_bass_kernel_spmd

    f1 = np.ascontiguousarray(np.asarray(feature1, dtype=np.float32))
    f2 = np.ascontiguousarray(np.asarray(feature2, dtype=np.float32))
    assert f1.shape == (N, D) and f2.shape == (N, D)

    bf16 = ml_dtypes.bfloat16
    fp8 = ml_dtypes.float8_e4m3
    sq1 = np.sum(f1.astype(np.float64) * f1, axis=1)
    sq2 = np.sum(f2.astype(np.float64) * f2, axis=1)

    nc, ops = _get_nc()

    # Sort feature2 rows by sq2 so per-group min-sq2 thresholds are tight.
    perm2 = np.argsort(sq2, kind="stable")
    f2s = f2[perm2]                                   # [N, D] fp32
    sq2s = sq2[perm2]
    minsq2 = sq2s.reshape(NG, BANKW).min(axis=1)      # per j-group min (fp64)

    # fp8 of 2*f2s with exact error norms
    f2q2 = (2.0 * f2s).astype(fp8)                    # [N, D]
    f2q2f = f2q2.astype(np.float32)
    dn2 = np.linalg.norm(2.0 * f2s.astype(np.float64) - f2q2f, axis=1)
    n2q = np.linalg.norm(f2q2f.astype(np.float64), axis=1)
    maxdn2 = dn2.reshape(NG, BANKW).max(axis=1)
    maxn2q = n2q.reshape(NG, BANKW).max(axis=1)

    # moving operand [64, 2g+i, j']: f2q2[g*512+j'][i*64+k]
    f2r = np.ascontiguousarray(
        f2q2.reshape(NG, BANKW, 2, 64).transpose(3, 0, 2, 1).reshape(64, NG * 2, BANKW)
    )

    # Shard feature1: global sq1 sort, stripe c::8 across cores; within a
    # core assign ascending local row lr -> tile t = lr % 8, partition
    # p = lr // 8, so partition p covers 8 consecutive sorted rows across
    # all tiles.
    perm1 = np.argsort(sq1, kind="stable")
    rowids = [perm1[c::NCORES] for c in range(NCORES)]

    # error-accumulation + threshold-rounding safety
    EPS_ACC = 0.05

    in_maps = []
    for c in range(NCORES):
        rid = rowids[c]
        f1c = f1[rid]                                  # [R, D] ascending sq1
        sq1c = sq1[rid]
        f1q = f1c.astype(fp8)
        f1qf = f1q.astype(np.float32)
        dn1 = np.linalg.norm(f1c.astype(np.float64) - f1qf, axis=1)
        nf1 = np.sqrt(sq1c)

        # stationary operand [64, i, t*128+m]: f1q[8m + t][i*64+k]
        f1l = np.ascontiguousarray(
            f1q.reshape(128, NT, 2, 64).transpose(3, 2, 1, 0).reshape(64, 2, NT * 128)
        )

        # per-op screen bias columns: screen term relu(z + bias) with
        # bias = -(min over covered (t,g) of [sq1 + minsq2 - 1 - margin])
        nd = sum(1 for e, _, _ in ops if e == "d")
        na = sum(1 for e, _, _ in ops if e == "a")
        biad = np.empty((128, max(nd, 1)), np.float64)
        bia = np.empty((128, max(na, 1)), np.float64)
        biad[:] = -3.0e38
        bia[:] = -3.0e38
        p = np.arange(128)
        i_d = 0
        i_a = 0
        for e, e0, el in ops:
            if e == "g":
                continue
            lim = None
            for B in range(e0 // BANKW, (e0 + el - 1) // BANKW + 1):
                t, g = B // NG, B % NG
                rows = 8 * p + t                       # local rows per partition
                cand = (
                    sq1c[rows] + minsq2[g] - 1.0 - EPS_ACC
                    - (nf1[rows] * maxdn2[g] + dn1[rows] * maxn2q[g])
                )
                lim = cand if lim is None else np.minimum(lim, cand)
            if e == "d":
                biad[:, i_d] = -lim
                i_d += 1
            else:
                bia[:, i_a] = -lim
                i_a += 1
        # conservative fp32 rounding: bias up (toward firing)
        biad32 = np.nextafter(biad.astype(np.float32), np.float32(3.0e38))
        bia32 = np.nextafter(bia.astype(np.float32), np.float32(3.0e38))

        diffb = (f1c - f2[rid]).reshape(128, R).astype(bf16)

        in_maps.append(
            {
                "f2r": f2r,
                "f1l": f1l,
                "biad": np.ascontiguousarray(biad32),
                "bia": np.ascontiguousarray(bia32),
                "diff": np.ascontiguousarray(diffb),
            }
        )

    res = run_bass_kernel_spmd(
        nc,
        in_maps,
        core_ids=list(range(NCORES)),
        trace=TRACE,
        **TRACE_KWARGS,
    )
    LAST_RESULT = res

    diag_total = np.float64(0.0)
    screen_total = np.float64(0.0)
    for r in res.results:
        out = r["out"]
        diag_total += np.float64(out[0, 0])
        screen_total += np.float64(out[1, 0])
    LAST_SCREEN = (diag_total, screen_total)

    if screen_total != 0.0:
        return _full_numpy_fallback(f1, f2)

    return np.float32(diag_total / (2.0 * N))


# revision 40
# speedup vs baseline: 1.1080x; 1.1080x over previous
"""L2 contrastive loss (margin=1.0) on 8 Trainium2 NeuronCores.

loss = (sum_{i!=j} relu(1 - d_ij)^2 + sum_i d_ii^2) / (2N),
d_ij = ||f1_i - f2_j||.

Strategy (certificate kernel): each core verifies on-device that every
pair in its 1024 x 8192 block of the distance matrix satisfies
d_ij >= 1 (so every hinge term is exactly 0) and computes its share of
the exact diagonal sum_i ||f1_i - f2_i||^2.  Host combines:
loss = diag/(2N) when every core's screen is zero, else falls back to
an exact host computation.

Device algorithm per core:
  * PE: z_ij = f1q_i . (2 f2q)_j in plain fp8e4m3 (1 cycle/col) into a
    [128, 3584] PSUM ring of 7 banks; bank 8 is a scratch target for
    keep-alive matmuls.  The PE p-state only reaches 2.4 GHz under
    continuous work, so the kernel primes the PE with dummy matmuls on
    memset data during the DMA lead-in and drops one dummy per ring
    wrap to keep the clock up while the screens (the real bottleneck)
    drain the ring.
  * Block skip (Cauchy-Schwarz): rows/cols are norm-sorted; a
    (i-tile, j-group) block whose norm intervals are separated by >= 1
    satisfies d2 >= (n1-n2)^2 >= 1 for every pair and is skipped on the
    host (no matmul, no screen).  feature1 rows are striped so every
    core has identical tile norm-bands -> one NEFF per skip pattern.
  * Screens drain the ring with both PSUM-capable engines concurrently
    (DVE tensor_scalar and ACT activation, each computing the sum of
    relu(z + bias) into per-op accumulator columns; the sum is zero iff
    every covered pair is certified outside the margin).  A DVE/ACT
    instruction reads PSUM at 1 elem/lane/cycle (0.96 + 1.2 GHz
    combined is the hard drain ceiling); op spans come from a host-side
    greedy planner that drifts around the ring so bank refills hide
    behind the opposite engine.
  * Certificate: relu(z + bias) == 0 iff z <= -bias; bias column =
    -(min over covered (tile, group) of [sq1_i + min sq2_g - 1 -
    margin]), margin a rigorous bound on the fp8 quantization error
    (|2 dot - z| <= n1*|dn2| + |dn1|*n2q, Cauchy-Schwarz with exactly-
    computed host-side error norms).  Ops may span two adjacent
    norm-sorted tiles, which keeps the min tight.
  * Diagonal: ACT Square+accum over host-computed bf16 (f1 - f2) rows.
Host: loss = sum(diag partials) / (2N) when every core's screen passes;
otherwise falls back to an exact full computation.
"""

import numpy as np
import ml_dtypes

N = 8192
D = 128
NCORES = 8
R = N // NCORES      # 1024 rows of feature1 per core
NT = 8               # i-tiles per core (128 rows each)
NG = 16              # j-groups (512 cols each)
NBANK = 7            # PSUM ring banks (bank 8 = PE keep-alive scratch)
BANKW = 512
RING = NBANK * BANKW
SCRATCH = RING       # scratch bank offset (elems)

TRACE = False       # test harness can set kernel.TRACE = True
TRACE_KWARGS = {}
LAST_RESULT = None  # BassKernelResults of the last run
LAST_SCREEN = None  # (diag_total, screen_total) of the last run

_BASS_CACHE = {}

N_PRIME = 10         # PE warm-up dummies during the DMA lead-in

# ---- screen planner ------------------------------------------------------
# Measured (ns): DVE tensor_scalar of W elems: W*1.0417 + ~175; ACT relu
# of L elems: L*0.8333 + ~420.  PE fills a bank in ~225ns at 2.4GHz.
PLAN_D_EL = 1472     # DVE op span (elems)
PLAN_A_EL = 1552     # ACT op span (elems)


def _plan(kept, d_el=None, a_el=None):
    """Greedy elem-granular ring schedule over the kept banks.  Bank k of
    the kept sequence holds elems [k*512, (k+1)*512) of the screened
    stream; spans never cross a ring-wrap boundary.  Returns (ops, est);
    op = (engine, e0, el)."""
    d_el = d_el or PLAN_D_EL
    a_el = a_el or PLAN_A_EL
    totel = len(kept) * BANKW
    dve_c = 1.0417
    act_c = 0.8333
    dve_oh = 175.0
    act_oh = 420.0
    sem = 250.0
    fill = 225.0
    eng_free = {"d": 0.0, "a": 3600.0}
    freed = [0.0] * NBANK
    filled = [0.0] * NBANK
    pe_t = 3400.0            # primes keep PE busy through the DMA lead-in
    next_fill = 0
    ops = []
    E = 0
    diag_done = False

    def fill_to(gbank):
        nonlocal pe_t, next_fill
        while next_fill <= gbank:
            b = next_fill % NBANK
            start = max(pe_t, freed[b] + sem) if next_fill >= NBANK else pe_t
            pe_t = start + fill
            filled[b] = pe_t
            next_fill += 1

    while E < totel:
        e = "d" if eng_free["d"] <= eng_free["a"] else "a"
        tgt = d_el if e == "d" else a_el
        el = min(tgt, RING - (E % RING), totel - E)
        b0, b1 = E // BANKW, (E + el - 1) // BANKW
        fill_to(b1)
        ready = max(filled[b % NBANK] for b in range(b0, b1 + 1))
        start = max(eng_free[e], ready + sem / 2)
        busy = el * dve_c + dve_oh if e == "d" else el * act_c + act_oh
        end = start + busy
        ops.append((e, E, el))
        eng_free[e] = end
        for b in range(b0, b1 + 1):
            freed[b % NBANK] = end
        E += el
        if not diag_done and e == "a" and eng_free["a"] > 7500.0:
            ops.append(("g", 0, R))
            eng_free["a"] += R * act_c + act_oh
            diag_done = True
    if not diag_done:
        ops.append(("g", 0, R))
        eng_free["a"] += R * act_c + act_oh
    return ops, max(eng_free.values())


def _build_bass(kept, ops):
    import concourse.bacc as bacc
    import concourse.mybir as mybir
    import concourse.tile as tile

    fp32 = mybir.dt.float32
    bf16 = mybir.dt.bfloat16
    fp8 = mybir.dt.float8e4
    Alu = mybir.AluOpType
    Act = mybir.ActivationFunctionType

    nd = sum(1 for e, _, _ in ops if e == "d")
    na = sum(1 for e, _, _ in ops if e == "a")
    max_d = max((el for e, _, el in ops if e == "d"), default=2)
    max_a = max((el for e, _, el in ops if e == "a"), default=2)

    nc = bacc.Bacc("TRN2", target_bir_lowering=False, debug=False,
                   num_devices=NCORES)

    # ---- DRAM I/O ----
    # fp8 of (2*f2_sorted).T  [D, N]
    d_f2t = nc.dram_tensor("f2t", [D, N], fp8, kind="ExternalInput")
    # fp8 of f1_core.T        [D, R] (tile t = cols [128t, 128t+128))
    d_f1t = nc.dram_tensor("f1t", [D, R], fp8, kind="ExternalInput")
    # per-op screen bias columns (DVE / ACT)
    d_biad = nc.dram_tensor("biad", [128, max(nd, 1)], fp32, kind="ExternalInput")
    d_bia = nc.dram_tensor("bia", [128, max(na, 1)], fp32, kind="ExternalInput")
    # bf16 host-computed (f1 - f2) rows for the exact diagonal
    d_diff = nc.dram_tensor("diff", [128, R], bf16, kind="ExternalInput")
    # out[0,0] = sum_i ||f1_i - f2_i||^2 ; out[1,0] = screen (0 iff no hinge)
    d_out = nc.dram_tensor("out", [2, 1], fp32, kind="ExternalOutput")

    # DMA chunking of f2t by column, sized so early banks unblock fast.
    # Kept banks are filled in kept-list order; chunk boundaries chosen
    # from the group of the 4th/12th kept bank.
    gs = [g for (_, g) in kept]
    c1 = max(gs[:5]) + 1 if len(gs) >= 5 else NG
    c2 = max(c1, (max(gs[:14]) + 1 if len(gs) >= 14 else NG))

    with tile.TileContext(nc) as tc:
        with (
            tc.tile_pool(name="singles", bufs=1) as singles,
            tc.tile_pool(name="chunks", bufs=1) as chunks,
        ):
            # ---- input DMAs (sync HWDGE ring is FIFO: order matters) ----
            s_f2t = chunks.tile([D, N], fp8, tag="f2t")
            nc.sync.dma_start(s_f2t[:, 0: c1 * BANKW], d_f2t[:, 0: c1 * BANKW])
            s_f1t = singles.tile([D, R], fp8, tag="f1t")
            nc.sync.dma_start(s_f1t[:, :], d_f1t[:, :])
            s_bia = singles.tile([128, max(na, 1)], fp32, tag="bia")
            nc.sync.dma_start(s_bia[:, :], d_bia[:, :])
            s_biad = singles.tile([128, max(nd, 1)], fp32, tag="biad")
            nc.sync.dma_start(s_biad[:, :], d_biad[:, :])
            if c2 > c1:
                nc.sync.dma_start(
                    s_f2t[:, c1 * BANKW: c2 * BANKW],
                    d_f2t[:, c1 * BANKW: c2 * BANKW],
                )
            s_diff = singles.tile([128, R], bf16, tag="diff_in")
            nc.sync.dma_start(s_diff[:, :], d_diff[:, :])
            if c2 < NG:
                nc.sync.dma_start(
                    s_f2t[:, c2 * BANKW:], d_f2t[:, c2 * BANKW:]
                )

            # ---- accumulators & trash ----
            acc_diag = singles.tile([128, 1], fp32, tag="acc_diag")
            acc_d = singles.tile([128, max(nd, 1)], fp32, tag="acc_d")
            acc_a = singles.tile([128, max(na, 1)], fp32, tag="acc_a")
            trash_d = singles.tile([128, max_d], bf16, tag="trash_d")
            trash_a = singles.tile([128, max_a], bf16, tag="trash_a")
            trash32 = singles.tile([128, R], fp32, tag="trash32")
            m_final = singles.tile([128, 2], fp32, tag="m_final")
            ones_sb = singles.tile([128, 1], fp32, tag="ones_sb")
            red_d = singles.tile([128, 1], fp32, tag="red_d")
            red_a = singles.tile([128, 1], fp32, tag="red_a")
            out_sb = singles.tile([2, 1], fp32, tag="out_sb")
            dummy = singles.tile([128, 640], fp8, tag="dummy")

            nc.vector.memset(ones_sb[:, :], 1.0)
            nc.vector.memset(dummy[:, :], 0.25)

            # ACT warmup: trigger the Relu act-table load during the DMA
            # lead-in so the first real screen doesn't pay ~2.7us for it.
            nc.scalar.activation(
                m_final[:, 0:1], ones_sb[:, :], Act.Relu, scale=1.0
            )

            # ---- main loop: PE ring fill + planned screens, emitted
            # interleaved in plan order ----
            with tc.tile_pool(name="psum_main", bufs=1, space="PSUM") as pp:
                big = pp.tile([128, RING + BANKW], fp32, tag="big")

                # PE p-state priming on memset data (into the scratch bank)
                for _ in range(N_PRIME):
                    nc.tensor.matmul(
                        big[:, SCRATCH: SCRATCH + BANKW],
                        lhsT=dummy[:, 0:128],
                        rhs=dummy[:, 128:640],
                        start=True,
                        stop=True,
                    )

                next_mm = 0

                def fill_banks_to(kbank):
                    nonlocal next_mm
                    while next_mm <= kbank:
                        t, g = kept[next_mm]
                        off = (next_mm % NBANK) * BANKW
                        nc.tensor.matmul(
                            big[:, off: off + BANKW],
                            lhsT=s_f1t[:, t * 128: (t + 1) * 128],
                            rhs=s_f2t[:, g * BANKW: (g + 1) * BANKW],
                            start=True,
                            stop=True,
                        )
                        next_mm += 1
                        if next_mm % NBANK == 0:
                            # keep-alive: PE never idles long enough to
                            # drop out of the 2.4GHz p-state
                            nc.tensor.matmul(
                                big[:, SCRATCH: SCRATCH + BANKW],
                                lhsT=s_f1t[:, t * 128: (t + 1) * 128],
                                rhs=s_f2t[:, g * BANKW: (g + 1) * BANKW],
                                start=True,
                                stop=True,
                            )

                i_d = 0
                i_a = 0
                for e, e0, el in ops:
                    if e == "g":
                        # exact diagonal: sum_i ||f1_i - f2_i||^2
                        nc.scalar.activation(
                            trash32[:, :],
                            s_diff[:, :],
                            Act.Square,
                            accum_out=acc_diag[:, 0:1],
                        )
                        continue
                    fill_banks_to((e0 + el - 1) // BANKW)
                    off = e0 % RING
                    if e == "d":
                        nc.vector.tensor_scalar(
                            trash_d[:, 0:el],
                            big[:, off: off + el],
                            s_biad[:, i_d: i_d + 1],
                            0.0,
                            Alu.add,
                            Alu.max,
                            accum_out=acc_d[:, i_d: i_d + 1],
                        )
                        i_d += 1
                    else:
                        nc.scalar.activation(
                            trash_a[:, 0:el],
                            big[:, off: off + el],
                            Act.Relu,
                            bias=s_bia[:, i_a: i_a + 1],
                            scale=1.0,
                            accum_out=acc_a[:, i_a: i_a + 1],
                        )
                        i_a += 1

            # ---- final reduction ----
            nc.vector.tensor_reduce(
                red_d[:, :], acc_d[:, :], axis=mybir.AxisListType.X, op=Alu.add
            )
            nc.vector.tensor_reduce(
                red_a[:, :], acc_a[:, :], axis=mybir.AxisListType.X, op=Alu.add
            )
            nc.vector.tensor_copy(m_final[:, 0:1], acc_diag[:, 0:1])
            nc.vector.tensor_add(m_final[:, 1:2], red_d[:, :], red_a[:, :])

            with tc.tile_pool(name="psum_fin", bufs=1, space="PSUM") as pf_pool:
                pf = pf_pool.tile([2, 1], fp32, tag="pf")
                nc.tensor.matmul(
                    pf[:, :], lhsT=m_final[:, :], rhs=ones_sb[:, :],
                    start=True, stop=True,
                )
                nc.vector.tensor_copy(out_sb[:, :], pf[:, :])

            nc.sync.dma_start(d_out[:, :], out_sb[:, :])

    nc.compile()
    return nc


def _get_nc(kept):
    key = tuple(kept)
    if key not in _BASS_CACHE:
        ops, _ = _plan(kept)
        _BASS_CACHE[key] = (_build_bass(kept, ops), ops)
    return _BASS_CACHE[key]


def _full_numpy_fallback(f1, f2):
    """Exact reference computation (only used if the screen certificate
    fails, i.e. some pair has d_ij close to or inside the margin)."""
    f1 = f1.astype(np.float32)
    f2 = f2.astype(np.float32)
    n = f1.shape[0]
    sq1 = np.sum(f1 * f1, axis=1)
    sq2 = np.sum(f2 * f2, axis=1)
    total = np.float64(0.0)
    chunk = 512
    for s in range(0, n, chunk):
        e = min(s + chunk, n)
        d2 = sq1[s:e, None] + sq2[None, :] - 2.0 * (f1[s:e] @ f2.T)
        d = np.sqrt(np.maximum(d2, 0.0))
        c = np.maximum(1.0 - d, 0.0)
        for r in range(s, e):
            c[r - s, r] = 0.0
        total += np.float64(np.sum(c * c))
    total += np.float64(np.sum((f1 - f2) ** 2))
    return np.float32(total / (2.0 * n))


def kernel(feature1, feature2):
    global LAST_RESULT, LAST_SCREEN
    from concourse.bass_utils import run_bass_kernel_spmd

    f1 = np.ascontiguousarray(np.asarray(feature1, dtype=np.float32))
    f2 = np.ascontiguousarray(np.asarray(feature2, dtype=np.float32))
    assert f1.shape == (N, D) and f2.shape == (N, D)

    bf16 = ml_dtypes.bfloat16
    fp8 = ml_dtypes.float8_e4m3
    sq1 = np.sum(f1.astype(np.float64) * f1, axis=1)
    sq2 = np.sum(f2.astype(np.float64) * f2, axis=1)

    # Sort feature2 rows by sq2; j-group g = sorted rows [512g, 512g+512).
    perm2 = np.argsort(sq2, kind="stable")
    f2s = f2[perm2]
    sq2s = sq2[perm2]
    minsq2 = sq2s.reshape(NG, BANKW).min(axis=1)

    # fp8 of 2*f2s with exact error norms
    f2q2 = (2.0 * f2s).astype(fp8)
    f2q2f = f2q2.astype(np.float32)
    dn2 = np.linalg.norm(2.0 * f2s.astype(np.float64) - f2q2f, axis=1)
    n2q = np.linalg.norm(f2q2f.astype(np.float64), axis=1)
    maxdn2 = dn2.reshape(NG, BANKW).max(axis=1)
    maxn2q = n2q.reshape(NG, BANKW).max(axis=1)
    f2t = np.ascontiguousarray(f2q2.T)                # [D, N] fp8

    # Shard feature1: global sq1 sort, stripe c::8.  Tile t of every core
    # covers global sorted ranks [1024t, 1024(t+1)) -> identical norm
    # bands -> one skip pattern (and one NEFF) for all cores.
    perm1 = np.argsort(sq1, kind="stable")
    sq1s = sq1[perm1]
    rowids = [perm1[c::NCORES] for c in range(NCORES)]

    # Cauchy-Schwarz block skip on exact norms: a (tile, group) block
    # with norm intervals separated by >= 1 has d2 >= (n1-n2)^2 >= 1.
    n1lo = np.sqrt(sq1s.reshape(NT, R)[:, 0])
    n1hi = np.sqrt(sq1s.reshape(NT, R)[:, -1])
    n2lo = np.sqrt(sq2s.reshape(NG, BANKW)[:, 0])
    n2hi = np.sqrt(sq2s.reshape(NG, BANKW)[:, -1])
    kept = []
    for t in range(NT):
        for g in range(NG):
            certified = (n2lo[g] - n1hi[t] >= 1.0 + 1e-6) or (
                n1lo[t] - n2hi[g] >= 1.0 + 1e-6
            )
            if not certified:
                kept.append((t, g))

    nc, ops = _get_nc(kept)

    # error-accumulation + threshold-rounding safety
    EPS_ACC = 0.05

    nd = sum(1 for e, _, _ in ops if e == "d")
    na = sum(1 for e, _, _ in ops if e == "a")

    in_maps = []
    for c in range(NCORES):
        rid = rowids[c]
        f1c = f1[rid]                                  # [R, D] ascending sq1
        sq1c = sq1[rid]
        f1q = f1c.astype(fp8)
        f1qf = f1q.astype(np.float32)
        dn1 = np.linalg.norm(f1c.astype(np.float64) - f1qf, axis=1)
        nf1 = np.sqrt(sq1c)
        f1t = np.ascontiguousarray(f1q.T)              # [D, R] fp8

        # per-op screen bias columns
        biad = np.full((128, max(nd, 1)), -3.0e38)
        bia = np.full((128, max(na, 1)), -3.0e38)
        p = np.arange(128)
        i_d = 0
        i_a = 0
        for e, e0, el in ops:
            if e == "g":
                continue
            lim = None
            for B in range(e0 // BANKW, (e0 + el - 1) // BANKW + 1):
                t, g = kept[B]
                rows = 128 * t + p                     # tile-major rows
                cand = (
                    sq1c[rows] + minsq2[g] - 1.0 - EPS_ACC
                    - (nf1[rows] * maxdn2[g] + dn1[rows] * maxn2q[g])
                )
                lim = cand if lim is None else np.minimum(lim, cand)
            if e == "d":
                biad[:, i_d] = -lim
                i_d += 1
            else:
                bia[:, i_a] = -lim
                i_a += 1
        # conservative fp32 rounding: bias up (toward firing)
        biad32 = np.nextafter(biad.astype(np.float32), np.float32(3.0e38))
        bia32 = np.nextafter(bia.astype(np.float32), np.float32(3.0e38))

        diffb = (f1c - f2[rid]).reshape(128, R).astype(bf16)

        in_maps.append(
            {
                "f2t": f2t,
                "f1t": f1t,
                "biad": np.ascontiguousarray(biad32),
                "bia": np.ascontiguousarray(bia32),
                "diff": np.ascontiguousarray(diffb),
            }
        )

    res = run_bass_kernel_spmd(
        nc,
        in_maps,
        core_ids=list(range(NCORES)),
        trace=TRACE,
        **TRACE_KWARGS,
    )
    LAST_RESULT = res

    diag_total = np.float64(0.0)
    screen_total = np.float64(0.0)
    for r in res.results:
        out = r["out"]
        diag_total += np.float64(out[0, 0])
        screen_total += np.float64(out[1, 0])
    LAST_SCREEN = (diag_total, screen_total)

    if screen_total != 0.0:
        return _full_numpy_fallback(f1, f2)

    return np.float32(diag_total / (2.0 * N))


# revision 41
# speedup vs baseline: 1.5459x; 1.3952x over previous
"""L2 contrastive loss (margin=1.0) on 8 Trainium2 NeuronCores.

loss = (sum_{i!=j} relu(1 - d_ij)^2 + sum_i d_ii^2) / (2N),
d_ij = ||f1_i - f2_j||.

Strategy (certificate kernel): each core verifies on-device that every
pair in its 1024 x 8192 block of the distance matrix satisfies
d_ij >= 1 (so every hinge term is exactly 0) and computes its share of
the exact diagonal sum_i ||f1_i - f2_i||^2.  Host combines:
loss = diag/(2N) when every core's screen is zero, else falls back to
an exact host computation.

Device algorithm per core:
  * PE: z_ij = f1q_i . (2 f2q)_j in plain fp8e4m3 (1 cycle/col) into a
    [128, 3584] PSUM ring of 7 banks; bank 8 is a scratch target for
    keep-alive matmuls.  The PE p-state only reaches 2.4 GHz under
    continuous work, so the kernel primes the PE with dummy matmuls on
    memset data during the DMA lead-in and drops one dummy per ring
    wrap to keep the clock up while the screens (the real bottleneck)
    drain the ring.
  * Block skip (Cauchy-Schwarz): rows/cols are norm-sorted; a
    (i-tile, j-group) block whose norm intervals are separated by >= 1
    satisfies d2 >= (n1-n2)^2 >= 1 for every pair and is skipped on the
    host (no matmul, no screen).  feature1 rows are striped so every
    core has identical tile norm-bands -> one NEFF per skip pattern.
  * Screens drain the ring with both PSUM-capable engines concurrently
    (DVE tensor_scalar and ACT activation, each computing the sum of
    relu(z + bias) into per-op accumulator columns; the sum is zero iff
    every covered pair is certified outside the margin).  A DVE/ACT
    instruction reads PSUM at 1 elem/lane/cycle (0.96 + 1.2 GHz
    combined is the hard drain ceiling); op spans come from a host-side
    greedy planner that drifts around the ring so bank refills hide
    behind the opposite engine.
  * Certificate: relu(z + bias) == 0 iff z <= -bias; bias column =
    -(min over covered (tile, group) of [sq1_i + min sq2_g - 1 -
    margin]), margin a rigorous bound on the fp8 quantization error
    (|2 dot - z| <= n1*|dn2| + |dn1|*n2q, Cauchy-Schwarz with exactly-
    computed host-side error norms).  Ops may span two adjacent
    norm-sorted tiles, which keeps the min tight.
  * Diagonal: ACT Square+accum over host-computed bf16 (f1 - f2) rows.
Host: loss = sum(diag partials) / (2N) when every core's screen passes;
otherwise falls back to an exact full computation.
"""

import numpy as np
import ml_dtypes

N = 8192
D = 128
NCORES = 8
R = N // NCORES      # 1024 rows of feature1 per core
NT = 8               # i-tiles per core (128 rows each)
NG = 16              # j-groups (512 cols each)
NBANK = 7            # PSUM ring banks (bank 8 = PE keep-alive scratch)
BANKW = 512
RING = NBANK * BANKW
SCRATCH = RING       # scratch bank offset (elems)

TRACE = False       # test harness can set kernel.TRACE = True
TRACE_KWARGS = {}
LAST_RESULT = None  # BassKernelResults of the last run
LAST_SCREEN = None  # (diag_total, screen_total) of the last run

_BASS_CACHE = {}

N_PRIME = 10         # PE warm-up dummies during the DMA lead-in

# ---- screen planner ------------------------------------------------------
# Measured (ns): DVE tensor_scalar of W elems: W*1.0417 + ~175; ACT relu
# of L elems: L*0.8333 + ~420.  PE fills a bank in ~225ns at 2.4GHz.
PLAN_D_EL = 1024     # DVE op span (elems)
PLAN_A_EL = 1024     # ACT op span (elems)


def _plan(kept, d_el=None, a_el=None):
    """Greedy elem-granular ring schedule over the kept banks.  Bank k of
    the kept sequence holds elems [k*512, (k+1)*512) of the screened
    stream; spans never cross a ring-wrap boundary.  Returns (ops, est);
    op = (engine, e0, el)."""
    d_el = d_el or PLAN_D_EL
    a_el = a_el or PLAN_A_EL
    totel = len(kept) * BANKW
    dve_c = 1.0417
    act_c = 0.8333
    dve_oh = 175.0
    act_oh = 420.0
    sem = 250.0
    fill = 225.0
    eng_free = {"d": 0.0, "a": 3600.0}
    freed = [0.0] * NBANK
    filled = [0.0] * NBANK
    pe_t = 3400.0            # primes keep PE busy through the DMA lead-in
    next_fill = 0
    ops = []
    E = 0
    diag_done = False

    def fill_to(gbank):
        nonlocal pe_t, next_fill
        while next_fill <= gbank:
            b = next_fill % NBANK
            start = max(pe_t, freed[b] + sem) if next_fill >= NBANK else pe_t
            pe_t = start + fill
            filled[b] = pe_t
            next_fill += 1

    while E < totel:
        e = "d" if eng_free["d"] <= eng_free["a"] else "a"
        tgt = d_el if e == "d" else a_el
        el = min(tgt, RING - (E % RING), totel - E)
        b0, b1 = E // BANKW, (E + el - 1) // BANKW
        fill_to(b1)
        ready = max(filled[b % NBANK] for b in range(b0, b1 + 1))
        start = max(eng_free[e], ready + sem / 2)
        busy = el * dve_c + dve_oh if e == "d" else el * act_c + act_oh
        end = start + busy
        ops.append((e, E, el))
        eng_free[e] = end
        for b in range(b0, b1 + 1):
            freed[b % NBANK] = end
        E += el
        if not diag_done and e == "a" and eng_free["a"] > 7500.0:
            ops.append(("g", 0, R))
            eng_free["a"] += R * act_c + act_oh
            diag_done = True
    if not diag_done:
        ops.append(("g", 0, R))
        eng_free["a"] += R * act_c + act_oh
    return ops, max(eng_free.values())


def _build_bass(kept, ops):
    import concourse.bacc as bacc
    import concourse.mybir as mybir
    import concourse.tile as tile

    fp32 = mybir.dt.float32
    bf16 = mybir.dt.bfloat16
    fp8 = mybir.dt.float8e4
    Alu = mybir.AluOpType
    Act = mybir.ActivationFunctionType

    nd = sum(1 for e, _, _ in ops if e == "d")
    na = sum(1 for e, _, _ in ops if e == "a")
    max_d = max((el for e, _, el in ops if e == "d"), default=2)
    max_a = max((el for e, _, el in ops if e == "a"), default=2)

    nc = bacc.Bacc("TRN2", target_bir_lowering=False, debug=False,
                   num_devices=NCORES)

    # ---- DRAM I/O ----
    # fp8 of (2*f2_sorted).T  [D, N]
    d_f2t = nc.dram_tensor("f2t", [D, N], fp8, kind="ExternalInput")
    # fp8 of f1_core.T        [D, R] (tile t = cols [128t, 128t+128))
    d_f1t = nc.dram_tensor("f1t", [D, R], fp8, kind="ExternalInput")
    # per-op screen bias columns (DVE / ACT)
    d_biad = nc.dram_tensor("biad", [128, max(nd, 1)], fp32, kind="ExternalInput")
    d_bia = nc.dram_tensor("bia", [128, max(na, 1)], fp32, kind="ExternalInput")
    # bf16 host-computed (f1 - f2) rows for the exact diagonal
    d_diff = nc.dram_tensor("diff", [128, R], bf16, kind="ExternalInput")
    # out[0,0] = sum_i ||f1_i - f2_i||^2 ; out[1,0] = screen (0 iff no hinge)
    d_out = nc.dram_tensor("out", [2, 1], fp32, kind="ExternalOutput")

    # DMA chunking of f2t by column, sized so early banks unblock fast.
    # Kept banks are filled in kept-list order; chunk boundaries chosen
    # from the group of the 4th/12th kept bank.
    gs = [g for (_, g) in kept]
    c1 = max(gs[:5]) + 1 if len(gs) >= 5 else NG
    c2 = max(c1, (max(gs[:14]) + 1 if len(gs) >= 14 else NG))

    with tile.TileContext(nc) as tc:
        with (
            tc.tile_pool(name="singles", bufs=1) as singles,
            tc.tile_pool(name="chunks", bufs=1) as chunks,
        ):
            # ---- input DMAs (sync HWDGE ring is FIFO: order matters) ----
            s_f2t = chunks.tile([D, N], fp8, tag="f2t")
            nc.sync.dma_start(s_f2t[:, 0: c1 * BANKW], d_f2t[:, 0: c1 * BANKW])
            s_f1t = singles.tile([D, R], fp8, tag="f1t")
            nc.sync.dma_start(s_f1t[:, :], d_f1t[:, :])
            s_bia = singles.tile([128, max(na, 1)], fp32, tag="bia")
            nc.sync.dma_start(s_bia[:, :], d_bia[:, :])
            s_biad = singles.tile([128, max(nd, 1)], fp32, tag="biad")
            nc.sync.dma_start(s_biad[:, :], d_biad[:, :])
            if c2 > c1:
                nc.sync.dma_start(
                    s_f2t[:, c1 * BANKW: c2 * BANKW],
                    d_f2t[:, c1 * BANKW: c2 * BANKW],
                )
            s_diff = singles.tile([128, R], bf16, tag="diff_in")
            nc.sync.dma_start(s_diff[:, :], d_diff[:, :])
            if c2 < NG:
                nc.sync.dma_start(
                    s_f2t[:, c2 * BANKW:], d_f2t[:, c2 * BANKW:]
                )

            # ---- accumulators & trash ----
            acc_diag = singles.tile([128, 1], fp32, tag="acc_diag")
            acc_d = singles.tile([128, max(nd, 1)], fp32, tag="acc_d")
            acc_a = singles.tile([128, max(na, 1)], fp32, tag="acc_a")
            trash_d = singles.tile([128, max_d], bf16, tag="trash_d")
            trash_a = singles.tile([128, max_a], bf16, tag="trash_a")
            trash32 = singles.tile([128, R], fp32, tag="trash32")
            m_final = singles.tile([128, 2], fp32, tag="m_final")
            ones_sb = singles.tile([128, 1], fp32, tag="ones_sb")
            red_d = singles.tile([128, 1], fp32, tag="red_d")
            red_a = singles.tile([128, 1], fp32, tag="red_a")
            out_sb = singles.tile([2, 1], fp32, tag="out_sb")
            dummy = singles.tile([128, 640], fp8, tag="dummy")

            nc.vector.memset(ones_sb[:, :], 1.0)
            nc.vector.memset(dummy[:, :], 0.25)

            # ACT warmup: trigger the Relu act-table load during the DMA
            # lead-in so the first real screen doesn't pay ~2.7us for it.
            nc.scalar.activation(
                m_final[:, 0:1], ones_sb[:, :], Act.Relu, scale=1.0
            )

            # ---- main loop: PE ring fill + planned screens, emitted
            # interleaved in plan order ----
            with tc.tile_pool(name="psum_main", bufs=1, space="PSUM") as pp:
                big = pp.tile([128, RING + BANKW], fp32, tag="big")

                # PE p-state priming on memset data (into the scratch bank)
                for _ in range(N_PRIME):
                    nc.tensor.matmul(
                        big[:, SCRATCH: SCRATCH + BANKW],
                        lhsT=dummy[:, 0:128],
                        rhs=dummy[:, 128:640],
                        start=True,
                        stop=True,
                    )

                next_mm = 0

                def fill_banks_to(kbank):
                    nonlocal next_mm
                    while next_mm <= kbank:
                        t, g = kept[next_mm]
                        off = (next_mm % NBANK) * BANKW
                        nc.tensor.matmul(
                            big[:, off: off + BANKW],
                            lhsT=s_f1t[:, t * 128: (t + 1) * 128],
                            rhs=s_f2t[:, g * BANKW: (g + 1) * BANKW],
                            start=True,
                            stop=True,
                        )
                        next_mm += 1
                        if next_mm % 4 == 0:
                            # keep-alive: PE never idles long enough to
                            # drop out of the 2.4GHz p-state
                            nc.tensor.matmul(
                                big[:, SCRATCH: SCRATCH + BANKW],
                                lhsT=s_f1t[:, t * 128: (t + 1) * 128],
                                rhs=s_f2t[:, g * BANKW: (g + 1) * BANKW],
                                start=True,
                                stop=True,
                            )

                i_d = 0
                i_a = 0
                for e, e0, el in ops:
                    if e == "g":
                        # exact diagonal: sum_i ||f1_i - f2_i||^2
                        nc.scalar.activation(
                            trash32[:, :],
                            s_diff[:, :],
                            Act.Square,
                            accum_out=acc_diag[:, 0:1],
                        )
                        continue
                    fill_banks_to((e0 + el - 1) // BANKW)
                    off = e0 % RING
                    if e == "d":
                        nc.vector.tensor_scalar(
                            trash_d[:, 0:el],
                            big[:, off: off + el],
                            s_biad[:, i_d: i_d + 1],
                            0.0,
                            Alu.add,
                            Alu.max,
                            accum_out=acc_d[:, i_d: i_d + 1],
                        )
                        i_d += 1
                    else:
                        nc.scalar.activation(
                            trash_a[:, 0:el],
                            big[:, off: off + el],
                            Act.Relu,
                            bias=s_bia[:, i_a: i_a + 1],
                            scale=1.0,
                            accum_out=acc_a[:, i_a: i_a + 1],
                        )
                        i_a += 1

            # ---- final reduction ----
            nc.vector.tensor_reduce(
                red_d[:, :], acc_d[:, :], axis=mybir.AxisListType.X, op=Alu.add
            )
            nc.vector.tensor_reduce(
                red_a[:, :], acc_a[:, :], axis=mybir.AxisListType.X, op=Alu.add
            )
            nc.vector.tensor_copy(m_final[:, 0:1], acc_diag[:, 0:1])
            nc.vector.tensor_add(m_final[:, 1:2], red_d[:, :], red_a[:, :])

            with tc.tile_pool(name="psum_fin", bufs=1, space="PSUM") as pf_pool:
                pf = pf_pool.tile([2, 1], fp32, tag="pf")
                nc.tensor.matmul(
                    pf[:, :], lhsT=m_final[:, :], rhs=ones_sb[:, :],
                    start=True, stop=True,
                )
                nc.vector.tensor_copy(out_sb[:, :], pf[:, :])

            nc.sync.dma_start(d_out[:, :], out_sb[:, :])

    nc.compile()
    return nc


def _get_nc(kept):
    key = tuple(kept)
    if key not in _BASS_CACHE:
        ops, _ = _plan(kept)
        _BASS_CACHE[key] = (_build_bass(kept, ops), ops)
    return _BASS_CACHE[key]


def _full_numpy_fallback(f1, f2):
    """Exact reference computation (only used if the screen certificate
    fails, i.e. some pair has d_ij close to or inside the margin)."""
    f1 = f1.astype(np.float32)
    f2 = f2.astype(np.float32)
    n = f1.shape[0]
    sq1 = np.sum(f1 * f1, axis=1)
    sq2 = np.sum(f2 * f2, axis=1)
    total = np.float64(0.0)
    chunk = 512
    for s in range(0, n, chunk):
        e = min(s + chunk, n)
        d2 = sq1[s:e, None] + sq2[None, :] - 2.0 * (f1[s:e] @ f2.T)
        d = np.sqrt(np.maximum(d2, 0.0))
        c = np.maximum(1.0 - d, 0.0)
        for r in range(s, e):
            c[r - s, r] = 0.0
        total += np.float64(np.sum(c * c))
    total += np.float64(np.sum((f1 - f2) ** 2))
    return np.float32(total / (2.0 * n))


def kernel(feature1, feature2):
    global LAST_RESULT, LAST_SCREEN
    from concourse.bass_utils import run_bass_kernel_spmd

    f1 = np.ascontiguousarray(np.asarray(feature1, dtype=np.float32))
    f2 = np.ascontiguousarray(np.asarray(feature2, dtype=np.float32))
    assert f1.shape == (N, D) and f2.shape == (N, D)

    bf16 = ml_dtypes.bfloat16
    fp8 = ml_dtypes.float8_e4m3
    sq1 = np.sum(f1.astype(np.float64) * f1, axis=1)
    sq2 = np.sum(f2.astype(np.float64) * f2, axis=1)

    # Sort feature2 rows by sq2; j-group g = sorted rows [512g, 512g+512).
    perm2 = np.argsort(sq2, kind="stable")
    f2s = f2[perm2]
    sq2s = sq2[perm2]
    minsq2 = sq2s.reshape(NG, BANKW).min(axis=1)

    # fp8 of 2*f2s with exact error norms
    f2q2 = (2.0 * f2s).astype(fp8)
    f2q2f = f2q2.astype(np.float32)
    dn2 = np.linalg.norm(2.0 * f2s.astype(np.float64) - f2q2f, axis=1)
    n2q = np.linalg.norm(f2q2f.astype(np.float64), axis=1)
    maxdn2 = dn2.reshape(NG, BANKW).max(axis=1)
    maxn2q = n2q.reshape(NG, BANKW).max(axis=1)
    f2t = np.ascontiguousarray(f2q2.T)                # [D, N] fp8

    # Shard feature1: global sq1 sort, stripe c::8.  Tile t of every core
    # covers global sorted ranks [1024t, 1024(t+1)) -> identical norm
    # bands -> one skip pattern (and one NEFF) for all cores.
    perm1 = np.argsort(sq1, kind="stable")
    sq1s = sq1[perm1]
    rowids = [perm1[c::NCORES] for c in range(NCORES)]

    # Cauchy-Schwarz block skip on exact norms: a (tile, group) block
    # with norm intervals separated by >= 1 has d2 >= (n1-n2)^2 >= 1.
    n1lo = np.sqrt(sq1s.reshape(NT, R)[:, 0])
    n1hi = np.sqrt(sq1s.reshape(NT, R)[:, -1])
    n2lo = np.sqrt(sq2s.reshape(NG, BANKW)[:, 0])
    n2hi = np.sqrt(sq2s.reshape(NG, BANKW)[:, -1])
    kept = []
    for t in range(NT):
        for g in range(NG):
            certified = (n2lo[g] - n1hi[t] >= 1.0 + 1e-6) or (
                n1lo[t] - n2hi[g] >= 1.0 + 1e-6
            )
            if not certified:
                kept.append((t, g))

    nc, ops = _get_nc(kept)

    # error-accumulation + threshold-rounding safety
    EPS_ACC = 0.05

    nd = sum(1 for e, _, _ in ops if e == "d")
    na = sum(1 for e, _, _ in ops if e == "a")

    in_maps = []
    for c in range(NCORES):
        rid = rowids[c]
        f1c = f1[rid]                                  # [R, D] ascending sq1
        sq1c = sq1[rid]
        f1q = f1c.astype(fp8)
        f1qf = f1q.astype(np.float32)
        dn1 = np.linalg.norm(f1c.astype(np.float64) - f1qf, axis=1)
        nf1 = np.sqrt(sq1c)
        f1t = np.ascontiguousarray(f1q.T)              # [D, R] fp8

        # per-op screen bias columns
        biad = np.full((128, max(nd, 1)), -3.0e38)
        bia = np.full((128, max(na, 1)), -3.0e38)
        p = np.arange(128)
        i_d = 0
        i_a = 0
        for e, e0, el in ops:
            if e == "g":
                continue
            lim = None
            for B in range(e0 // BANKW, (e0 + el - 1) // BANKW + 1):
                t, g = kept[B]
                rows = 128 * t + p                     # tile-major rows
                cand = (
                    sq1c[rows] + minsq2[g] - 1.0 - EPS_ACC
                    - (nf1[rows] * maxdn2[g] + dn1[rows] * maxn2q[g])
                )
                lim = cand if lim is None else np.minimum(lim, cand)
            if e == "d":
                biad[:, i_d] = -lim
                i_d += 1
            else:
                bia[:, i_a] = -lim
                i_a += 1
        # conservative fp32 rounding: bias up (toward firing)
        biad32 = np.nextafter(biad.astype(np.float32), np.float32(3.0e38))
        bia32 = np.nextafter(bia.astype(np.float32), np.float32(3.0e38))

        diffb = (f1c - f2[rid]).reshape(128, R).astype(bf16)

        in_maps.append(
            {
                "f2t": f2t,
                "f1t": f1t,
                "biad": np.ascontiguousarray(biad32),
                "bia": np.ascontiguousarray(bia32),
                "diff": np.ascontiguousarray(diffb),
            }
        )

    res = run_bass_kernel_spmd(
        nc,
        in_maps,
        core_ids=list(range(NCORES)),
        trace=TRACE,
        **TRACE_KWARGS,
    )
    LAST_RESULT = res

    diag_total = np.float64(0.0)
    screen_total = np.float64(0.0)
    for r in res.results:
        out = r["out"]
        diag_total += np.float64(out[0, 0])
        screen_total += np.float64(out[1, 0])
    LAST_SCREEN = (diag_total, screen_total)

    if screen_total != 0.0:
        return _full_numpy_fallback(f1, f2)

    return np.float32(diag_total / (2.0 * N))


# revision 42
# speedup vs baseline: 1.7845x; 1.1543x over previous
"""L2 contrastive loss (margin=1.0) on 8 Trainium2 NeuronCores.

loss = (sum_{i!=j} relu(1 - d_ij)^2 + sum_i d_ii^2) / (2N),
d_ij = ||f1_i - f2_j||.

Sharding: feature1 rows are globally sorted by squared norm and striped
across the 8 cores (core c gets sorted rows c::8), so every core's
i-tiles cover identical norm-quantile bands; every core sees all of
feature2 (sorted by squared norm) and handles a 1024 x 8192 block of
the distance matrix.

Block skip (Cauchy-Schwarz): a span whose f1-tile and f2-group norm
intervals are separated by >= 1 satisfies d2 >= (n1-n2)^2 >= 1 for every
pair, so it is certified hinge-free on the host and emitted neither as
matmuls nor screens.  The NEFF is built per skip-pattern (cached).

Device algorithm per core:
  * PE (bf16): psum = 2 * f1_i . f2_j, N=512 matmuls into a single
    [128 x 4096] PSUM tile (all 8 banks) used as 4 circular 1024-wide
    units, so the PE fills ahead while older units are screened.
  * Screen: every element is passed through
        relu(psum + (1 - sq1_i - min_tile sq2_j))
    with the per-partition bias column precomputed on host (feature2 is
    sorted by sq2 so the per-tile min is tight).  Since
    psum + bias >= 2dot + 1 - sq1_i - sq2_j = 1 - d2_ij, the accumulated
    screen is a CONSERVATIVE certificate: screen == 0  ==>  every
    d2_ij >= 1  ==>  every hinge term relu(1 - d_ij) is exactly 0.
    Screens alternate between DVE (tensor_scalar max+accum) and ACT
    (Relu + bias AP + accum) so both engines run concurrently on
    different PSUM units; both are saturated at their 1 elem/lane/cycle
    PSUM read rate, which is the binding resource of this kernel.
  * Diagonal: sum_i ||f1_i - f2_i||^2 in fp32 from host-precomputed
    (f1 - f2) rows (one ACT Square + accumulate, overlapped with the
    main loop), reduced to a scalar with a ones-matmul.
Host: loss = sum(diag partials) / (2N) when every core's screen is 0;
otherwise (only if some pair sits within/near the margin) falls back to
an exact full computation.
"""

import numpy as np
import ml_dtypes

N = 8192
D = 128
NCORES = 8
R = N // NCORES  # 1024 rows of feature1 per core

TRACE = False       # test harness can set kernel.TRACE = True
TRACE_KWARGS = {}
LAST_RESULT = None  # BassKernelResults of the last run

_BASS_CACHE = {}

# Span layout: 8 i-tiles x 8 j-groups of 1024 -> 64 spans.  PSUM holds a
# single [128, 4096] tile used as 4 circular 1024-wide units; screens
# alternate between DVE and ACT per span.
N_SUPER = 64
NJH = 8
JW = N // NJH  # 1024 j-columns per span


def _build_bass(keep):
    import concourse.bacc as bacc
    import concourse.mybir as mybir
    import concourse.tile as tile

    fp32 = mybir.dt.float32
    bf16 = mybir.dt.bfloat16
    Alu = mybir.AluOpType
    Act = mybir.ActivationFunctionType

    nc = bacc.Bacc("TRN2", target_bir_lowering=False, debug=False,
                   num_devices=NCORES)

    # ---- DRAM I/O ----
    # (2*f2_sorted).T in bf16 -- main matmul moving operand
    d_f2t2 = nc.dram_tensor("f2t2", [D, N], bf16, kind="ExternalInput")
    # f1_core.T in bf16 -- main matmul stationary operand
    d_f1t = nc.dram_tensor("f1t", [D, R], bf16, kind="ExternalInput")
    n_kept = sum(1 for m in keep if m)
    # screen bias columns [128, n_kept]: col k (kept-span order) holds
    # 1 - sq1[tile row p] - min_{j in span} sq2_j
    d_s1c = nc.dram_tensor("s1c", [128, n_kept], fp32, kind="ExternalInput")
    # fp32 host-computed (f1 - f2) rows for the exact diagonal
    d_diff = nc.dram_tensor("diff", [128, R], fp32, kind="ExternalInput")
    # out[0,0] = sum_i ||f1_i - f2_i||^2 ; out[1,0] = screen (0 iff no hinge)
    d_out = nc.dram_tensor("out", [2, 1], fp32, kind="ExternalOutput")

    with tile.TileContext(nc) as tc:
        with (
            tc.tile_pool(name="singles", bufs=1) as singles,
            tc.tile_pool(name="chunks", bufs=1) as chunks,
        ):
            # ---- input DMAs.  The sync HWDGE ring is FIFO, so order
            # matters: the first matmuls gate on chunk 0.
            CHUNK_COLS = [1024, 3072, 4096]
            s_cs = []
            bounds = []
            lo = 0
            for k, w in enumerate(CHUNK_COLS):
                ck = chunks.tile([D, w], bf16, tag=f"f2t2_{k}")
                s_cs.append(ck)
                bounds.append((lo, lo + w))
                lo += w
            # sync HWDGE ring is FIFO: chunk0 (gates the first matmuls)
            # goes first, bulk last.
            nc.sync.dma_start(s_cs[0][:, :], d_f2t2[:, bounds[0][0] : bounds[0][1]])
            s_f1t = singles.tile([D, R], bf16, tag="f1t")
            nc.sync.dma_start(s_f1t[:, :], d_f1t[:, :])
            s_s1c = singles.tile([128, n_kept], fp32, tag="s1c")
            nc.sync.dma_start(s_s1c[:, :], d_s1c[:, :])
            s_diff = singles.tile([128, R], fp32, tag="diff_in")
            nc.sync.dma_start(s_diff[:, :], d_diff[:, :])
            for k in (1, 2):
                nc.sync.dma_start(
                    s_cs[k][:, :], d_f2t2[:, bounds[k][0] : bounds[k][1]]
                )

            def f2t2_slice(jh, js):
                lo = jh * JW + js * 512
                for t, (a, b) in zip(s_cs, bounds):
                    if a <= lo < b:
                        return t[:, lo - a : lo - a + 512]
                raise AssertionError

            # ---- accumulators & trash ----
            acc_diag = singles.tile([128, 1], fp32, tag="acc_diag")
            acc_d = singles.tile([128, max(n_kept, 1)], fp32, tag="acc_d")
            acc_a = singles.tile([128, max(n_kept, 1)], fp32, tag="acc_a")
            n_units = 4096 // JW
            trash_d = singles.tile([128, JW], bf16, tag="trash_d")
            trash_a = singles.tile([128, JW], bf16, tag="trash_a")
            trash32 = singles.tile([128, R], fp32, tag="trash32")
            m_final = singles.tile([128, 2], fp32, tag="m_final")
            ones_sb = singles.tile([128, 1], fp32, tag="ones_sb")
            red_d = singles.tile([128, 1], fp32, tag="red_d")
            red_a = singles.tile([128, 1], fp32, tag="red_a")
            out_sb = singles.tile([2, 1], fp32, tag="out_sb")

            nc.vector.memset(ones_sb[:, :], 1.0)

            # ---- exact diagonal: sum_i ||f1_i - f2_i||^2 (fp32); runs
            # early on ACT, overlapped with the main loop ----
            nc.scalar.activation(
                trash32[:, :],
                s_diff[:, :],
                Act.Square,
                accum_out=acc_diag[:, 0:1],
            )

            # ---- main loop ----
            # One [128, 4096] PSUM tile = all 8 banks, used as 4 circular
            # 1024-wide units.  PE fills unit (st % 4) while earlier units
            # are screened; screens alternate DVE (even st) / ACT (odd st)
            # so both engines run concurrently on different units.
            order = [
                (ti, jh, keep[ti * NJH + jh])
                for ti in range(NCORES)
                for jh in range(NJH)
                if keep[ti * NJH + jh]
            ]
            # Greedy DVE/ACT assignment by measured per-op cost so the
            # mixed 512/1024-wide screens stay balanced across engines.
            def op_cost(fd, eng):
                if eng == "dve":
                    return 216.0 + fd / 0.96 + 263.0
                return 216.0 + fd / 1.2 + 583.0

            busy = {"dve": 0.0, "act": 0.0}
            engine_of = []
            for _, _, mode in order:
                fd = 512 * bin(mode).count("1")
                pick = min(("dve", "act"),
                           key=lambda e: busy[e] + op_cost(fd, e))
                engine_of.append(pick)
                busy[pick] += op_cost(fd, pick)

            with tc.tile_pool(name="psum_main", bufs=1, space="PSUM") as pp:
                big = pp.tile([128, 4096], fp32, tag="big")
                i_d = 0
                i_a = 0
                for st, (ti, jh, mode) in enumerate(order):
                    isl = slice(ti * 128, (ti + 1) * 128)
                    half = (st % n_units) * JW
                    # main matmuls for the kept 512-halves, packed from the
                    # unit start: mode 1 = lo half, 2 = hi half, 3 = both
                    halves = {1: (0,), 2: (1,), 3: (0, 1)}[mode]
                    for k, hv in enumerate(halves):
                        nc.tensor.matmul(
                            big[:, half + k * 512 : half + (k + 1) * 512],
                            lhsT=s_f1t[:, isl],
                            rhs=f2t2_slice(jh, hv),
                            start=True,
                            stop=True,
                        )
                    fd = 512 * len(halves)
                    # screen: relu(psum + bias_col) accumulated; zero iff
                    # no hinge term among the screened columns.
                    bias_col = s_s1c[:, st : st + 1]
                    if engine_of[st] == "dve":
                        nc.vector.tensor_scalar(
                            trash_d[:, 0:fd],
                            big[:, half : half + fd],
                            bias_col,
                            0.0,
                            Alu.add,
                            Alu.max,
                            accum_out=acc_d[:, i_d : i_d + 1],
                        )
                        i_d += 1
                    else:
                        nc.scalar.activation(
                            trash_a[:, 0:fd],
                            big[:, half : half + fd],
                            Act.Relu,
                            bias=bias_col,
                            scale=1.0,
                            accum_out=acc_a[:, i_a : i_a + 1],
                        )
                        i_a += 1

            # ---- final reduction ----
            nc.vector.tensor_reduce(
                red_d[:, :], acc_d[:, :], axis=mybir.AxisListType.X, op=Alu.add
            )
            nc.vector.tensor_reduce(
                red_a[:, :], acc_a[:, :], axis=mybir.AxisListType.X, op=Alu.add
            )
            nc.vector.tensor_copy(m_final[:, 0:1], acc_diag[:, 0:1])
            nc.vector.tensor_add(m_final[:, 1:2], red_d[:, :], red_a[:, :])

            with tc.tile_pool(name="psum_fin", bufs=1, space="PSUM") as pf_pool:
                pf = pf_pool.tile([2, 1], fp32, tag="pf")
                nc.tensor.matmul(
                    pf[:, :], lhsT=m_final[:, :], rhs=ones_sb[:, :],
                    start=True, stop=True,
                )
                nc.vector.tensor_copy(out_sb[:, :], pf[:, :])

            nc.sync.dma_start(d_out[:, :], out_sb[:, :])

    nc.compile()
    return nc


def _get_nc(keep):
    keep = tuple(bool(k) for k in keep)
    if keep not in _BASS_CACHE:
        _BASS_CACHE[keep] = _build_bass(keep)
    return _BASS_CACHE[keep]


def _full_numpy_fallback(f1, f2):
    """Exact reference computation (only used if the screen certificate
    fails, i.e. some pair has d_ij close to or inside the margin)."""
    f1 = f1.astype(np.float32)
    f2 = f2.astype(np.float32)
    n = f1.shape[0]
    sq1 = np.sum(f1 * f1, axis=1)
    sq2 = np.sum(f2 * f2, axis=1)
    total = np.float64(0.0)
    chunk = 512
    for s in range(0, n, chunk):
        e = min(s + chunk, n)
        d2 = sq1[s:e, None] + sq2[None, :] - 2.0 * (f1[s:e] @ f2.T)
        d = np.sqrt(np.maximum(d2, 0.0))
        c = np.maximum(1.0 - d, 0.0)
        for r in range(s, e):
            c[r - s, r] = 0.0
        total += np.float64(np.sum(c * c))
    total += np.float64(np.sum((f1 - f2) ** 2))
    return np.float32(total / (2.0 * n))


def kernel(feature1, feature2):
    global LAST_RESULT
    from concourse.bass_utils import run_bass_kernel_spmd

    f1 = np.ascontiguousarray(np.asarray(feature1, dtype=np.float32))
    f2 = np.ascontiguousarray(np.asarray(feature2, dtype=np.float32))
    assert f1.shape == (N, D) and f2.shape == (N, D)

    bf16 = ml_dtypes.bfloat16
    sq1 = np.sum(f1.astype(np.float64) * f1, axis=1)
    sq2 = np.sum(f2.astype(np.float64) * f2, axis=1)

    # Sort feature2 rows by sq2 so the per-supertile min-sq2 bias is tight.
    perm = np.argsort(sq2, kind="stable")
    f2s = f2[perm]
    sq2s = sq2[perm]
    sq2min = sq2s.reshape(NJH, JW).min(axis=1)  # per j-group minimum
    sq2max = sq2s.reshape(NJH, JW).max(axis=1)

    f2t2 = np.ascontiguousarray((2.0 * f2s.T).astype(bf16))           # [D, N]

    # Shard feature1 by striping the globally-sq1-sorted rows (core c gets
    # sorted rows c::8) so every core's i-tile ti covers the same norm
    # quantile band and the block-skip pattern is core-invariant.
    perm1 = np.argsort(sq1, kind="stable")
    rowids = [perm1[c::NCORES] for c in range(NCORES)]

    # Cauchy-Schwarz block certificate: a span (ti, jh) needs no screening
    # if |norm(f1_i) - norm(f2_j)| >= 1 for all pairs, i.e. the norm
    # intervals are separated by >= 1 (then d2 >= (n1-n2)^2 >= 1 exactly).
    # per-512-group norm intervals (sq2s ascending -> min is first elem)
    g2min = sq2s.reshape(16, 512).min(axis=1)
    g2max = sq2s.reshape(16, 512).max(axis=1)
    keep = []
    for ti in range(R // 128):
        n1lo = np.sqrt(min(sq1[rowids[c][ti * 128]] for c in range(NCORES)))
        n1hi = np.sqrt(max(sq1[rowids[c][(ti + 1) * 128 - 1]]
                           for c in range(NCORES)))
        for jh in range(NJH):
            mode = 0
            for hv in (0, 1):
                g = jh * 2 + hv
                n2lo, n2hi = np.sqrt(g2min[g]), np.sqrt(g2max[g])
                certified = (n2lo - n1hi >= 1.0 + 1e-6) or (
                    n1lo - n2hi >= 1.0 + 1e-6
                )
                if not certified:
                    mode |= 1 << hv
            keep.append(mode)
    kept_idx = [k for k, m in enumerate(keep) if m]

    in_maps = []
    for c in range(NCORES):
        rid = rowids[c]
        f1c_rows = f1[rid]                                            # [R, D]
        sq1c = sq1[rid]
        s1c = np.empty((128, len(kept_idx)), np.float32)
        for col, k in enumerate(kept_idx):
            ti, jh = k // NJH, k % NJH
            first_half = 0 if (keep[k] & 1) else 1
            s1c[:, col] = (
                1.0
                - sq1c[ti * 128 : (ti + 1) * 128]
                - g2min[jh * 2 + first_half]
            )
        in_maps.append(
            {
                "f2t2": f2t2,
                "f1t": np.ascontiguousarray(f1c_rows.T.astype(bf16)),
                "s1c": np.ascontiguousarray(s1c),
                "diff": np.ascontiguousarray(
                    f1c_rows.reshape(128, R) - f2[rid].reshape(128, R)
                ),
            }
        )

    nc = _get_nc(keep)
    res = run_bass_kernel_spmd(
        nc,
        in_maps,
        core_ids=list(range(NCORES)),
        trace=TRACE,
        **TRACE_KWARGS,
    )
    LAST_RESULT = res

    diag_total = np.float64(0.0)
    screen_total = np.float64(0.0)
    for r in res.results:
        out = r["out"]
        diag_total += np.float64(out[0, 0])
        screen_total += np.float64(out[1, 0])

    if screen_total != 0.0:
        return _full_numpy_fallback(f1, f2)

    return np.float32(diag_total / (2.0 * N))



# revision 48
# speedup vs baseline: 1.9677x; 1.1027x over previous
"""L2 contrastive loss (margin=1.0) on 8 Trainium2 NeuronCores.

loss = (sum_{i!=j} relu(1 - d_ij)^2 + sum_i d_ii^2) / (2N),
d_ij = ||f1_i - f2_j||.

Sharding: feature1 rows are globally sorted by squared norm and striped
across the 8 cores (core c gets sorted rows c::8), so every core's
i-tiles cover identical norm-quantile bands; every core sees all of
feature2 (sorted by squared norm) and handles a 1024 x 8192 block of
the distance matrix.

Block skip (Cauchy-Schwarz): a span whose f1-tile and f2-group norm
intervals are separated by >= 1 satisfies d2 >= (n1-n2)^2 >= 1 for every
pair, so it is certified hinge-free on the host and emitted neither as
matmuls nor screens.  The NEFF is built per skip-pattern (cached).

Device algorithm per core:
  * PE (bf16): psum = 2 * f1_i . f2_j, N=512 matmuls into a single
    [128 x 4096] PSUM tile (all 8 banks) used as 4 circular 1024-wide
    units, so the PE fills ahead while older units are screened.
  * Screen: every element is passed through
        relu(psum + (1 - sq1_i - min_tile sq2_j))
    with the per-partition bias column precomputed on host (feature2 is
    sorted by sq2 so the per-tile min is tight).  Since
    psum + bias >= 2dot + 1 - sq1_i - sq2_j = 1 - d2_ij, the accumulated
    screen is a CONSERVATIVE certificate: screen == 0  ==>  every
    d2_ij >= 1  ==>  every hinge term relu(1 - d_ij) is exactly 0.
    Screens alternate between DVE (tensor_scalar max+accum) and ACT
    (Relu + bias AP + accum) so both engines run concurrently on
    different PSUM units; both are saturated at their 1 elem/lane/cycle
    PSUM read rate, which is the binding resource of this kernel.
  * Diagonal: sum_i ||f1_i - f2_i||^2 in fp32 from host-precomputed
    (f1 - f2) rows (one ACT Square + accumulate, overlapped with the
    main loop), reduced to a scalar with a ones-matmul.
Host: loss = sum(diag partials) / (2N) when every core's screen is 0;
otherwise (only if some pair sits within/near the margin) falls back to
an exact full computation.
"""

import numpy as np
import ml_dtypes

N = 8192
D = 128
NCORES = 8
R = N // NCORES  # 1024 rows of feature1 per core

TRACE = False       # test harness can set kernel.TRACE = True
TRACE_KWARGS = {}
LAST_RESULT = None  # BassKernelResults of the last run

_BASS_CACHE = {}

# Span layout: 8 i-tiles x 8 j-groups of 1024 -> 64 spans.  PSUM holds a
# single [128, 4096] tile used as 4 circular 1024-wide units; screens
# alternate between DVE and ACT per span.
N_SUPER = 64
NJH = 8
JW = N // NJH  # 1024 j-columns per span


def _build_bass(keep):
    import concourse.bacc as bacc
    import concourse.mybir as mybir
    import concourse.tile as tile

    fp32 = mybir.dt.float32
    bf16 = mybir.dt.bfloat16
    Alu = mybir.AluOpType
    Act = mybir.ActivationFunctionType

    nc = bacc.Bacc("TRN2", target_bir_lowering=False, debug=False,
                   num_devices=NCORES)

    fp8 = mybir.dt.float8e4

    # ---- DRAM I/O ----
    # (2*f2_sorted).T in fp8 -- main matmul moving operand (same PE rate
    # as bf16, half the HBM traffic; quantization error is folded into
    # the screen bias columns as a rigorous Cauchy-Schwarz margin)
    d_f2t2 = nc.dram_tensor("f2t2", [D, N], fp8, kind="ExternalInput")
    # f1_core.T in fp8 -- main matmul stationary operand
    d_f1t = nc.dram_tensor("f1t", [D, R], fp8, kind="ExternalInput")
    n_kept = sum(1 for m in keep if m)
    # screen bias columns [128, n_kept]: col k (kept-span order) holds
    # 1 - sq1[tile row p] - min_{j in span} sq2_j
    d_s1c = nc.dram_tensor("s1c", [128, n_kept], fp32, kind="ExternalInput")
    # bf16 host-computed (f1 - f2) rows for the exact diagonal
    d_diff = nc.dram_tensor("diff", [128, R], bf16, kind="ExternalInput")
    # out[0,0] = sum_i ||f1_i - f2_i||^2 ; out[1,0] = screen (0 iff no hinge)
    d_out = nc.dram_tensor("out", [2, 1], fp32, kind="ExternalOutput")

    with tile.TileContext(nc) as tc:
        with (
            tc.tile_pool(name="singles", bufs=1) as singles,
            tc.tile_pool(name="chunks", bufs=1) as chunks,
        ):
            # ---- input DMAs.  The sync HWDGE ring is FIFO, so order
            # matters: the first matmuls gate on chunk 0.
            CHUNK_COLS = [1024, 3072, 4096]
            s_cs = []
            bounds = []
            lo = 0
            for k, w in enumerate(CHUNK_COLS):
                ck = chunks.tile([D, w], fp8, tag=f"f2t2_{k}")
                s_cs.append(ck)
                bounds.append((lo, lo + w))
                lo += w
            # sync HWDGE ring is FIFO: chunk0 (gates the first matmuls)
            # goes first, bulk last.
            nc.sync.dma_start(s_cs[0][:, :], d_f2t2[:, bounds[0][0] : bounds[0][1]])
            s_f1t = singles.tile([D, R], fp8, tag="f1t")
            nc.sync.dma_start(s_f1t[:, :], d_f1t[:, :])
            s_s1c = singles.tile([128, n_kept], fp32, tag="s1c")
            nc.sync.dma_start(s_s1c[:, :], d_s1c[:, :])
            s_diff = singles.tile([128, R], bf16, tag="diff_in")
            nc.sync.dma_start(s_diff[:, :], d_diff[:, :])
            for k in (1, 2):
                nc.sync.dma_start(
                    s_cs[k][:, :], d_f2t2[:, bounds[k][0] : bounds[k][1]]
                )

            def f2t2_slice(jh, js):
                lo = jh * JW + js * 512
                for t, (a, b) in zip(s_cs, bounds):
                    if a <= lo < b:
                        return t[:, lo - a : lo - a + 512]
                raise AssertionError

            # ---- accumulators & trash ----
            acc_diag = singles.tile([128, 1], fp32, tag="acc_diag")
            acc_d = singles.tile([128, max(n_kept, 1)], fp32, tag="acc_d")
            acc_a = singles.tile([128, max(n_kept, 1)], fp32, tag="acc_a")
            n_units = 4096 // JW
            trash_d = singles.tile([128, JW], bf16, tag="trash_d")
            trash_a = singles.tile([128, JW], bf16, tag="trash_a")
            trash32 = singles.tile([128, R], fp32, tag="trash32")
            m_final = singles.tile([128, 2], fp32, tag="m_final")
            ones_sb = singles.tile([128, 1], fp32, tag="ones_sb")
            red_d = singles.tile([128, 1], fp32, tag="red_d")
            red_a = singles.tile([128, 1], fp32, tag="red_a")
            out_sb = singles.tile([2, 1], fp32, tag="out_sb")

            nc.vector.memset(ones_sb[:, :], 1.0)

            # ---- exact diagonal: sum_i ||f1_i - f2_i||^2 (fp32); runs
            # early on ACT, overlapped with the main loop ----
            nc.scalar.activation(
                trash32[:, :],
                s_diff[:, :],
                Act.Square,
                accum_out=acc_diag[:, 0:1],
            )

            # ---- main loop ----
            # One [128, 4096] PSUM tile = all 8 banks, used as 4 circular
            # 1024-wide units.  PE fills unit (st % 4) while earlier units
            # are screened; screens alternate DVE (even st) / ACT (odd st)
            # so both engines run concurrently on different units.
            order = [
                (ti, jh, keep[ti * NJH + jh])
                for ti in range(NCORES)
                for jh in range(NJH)
                if keep[ti * NJH + jh]
            ]
            # Greedy DVE/ACT assignment by measured per-op cost so the
            # mixed 512/1024-wide screens stay balanced across engines.
            def op_cost(fd, eng):
                if eng == "dve":
                    return 216.0 + fd / 0.96 + 263.0
                return 216.0 + fd / 1.2 + 583.0

            busy = {"dve": 0.0, "act": 0.0}
            engine_of = []
            for _, _, mode in order:
                fd = 512 * bin(mode).count("1")
                pick = min(("dve", "act"),
                           key=lambda e: busy[e] + op_cost(fd, e))
                engine_of.append(pick)
                busy[pick] += op_cost(fd, pick)

            with tc.tile_pool(name="psum_main", bufs=1, space="PSUM") as pp:
                big = pp.tile([128, 4096], fp32, tag="big")
                i_d = 0
                i_a = 0
                for st, (ti, jh, mode) in enumerate(order):
                    isl = slice(ti * 128, (ti + 1) * 128)
                    half = (st % n_units) * JW
                    # main matmuls for the kept 512-halves, packed from the
                    # unit start: mode 1 = lo half, 2 = hi half, 3 = both
                    halves = {1: (0,), 2: (1,), 3: (0, 1)}[mode]
                    for k, hv in enumerate(halves):
                        nc.tensor.matmul(
                            big[:, half + k * 512 : half + (k + 1) * 512],
                            lhsT=s_f1t[:, isl],
                            rhs=f2t2_slice(jh, hv),
                            start=True,
                            stop=True,
                        )
                    fd = 512 * len(halves)
                    # screen: relu(psum + bias_col) accumulated; zero iff
                    # no hinge term among the screened columns.
                    bias_col = s_s1c[:, st : st + 1]
                    if engine_of[st] == "dve":
                        nc.vector.tensor_scalar(
                            trash_d[:, 0:fd],
                            big[:, half : half + fd],
                            bias_col,
                            0.0,
                            Alu.add,
                            Alu.max,
                            accum_out=acc_d[:, i_d : i_d + 1],
                        )
                        i_d += 1
                    else:
                        nc.scalar.activation(
                            trash_a[:, 0:fd],
                            big[:, half : half + fd],
                            Act.Relu,
                            bias=bias_col,
                            scale=1.0,
                            accum_out=acc_a[:, i_a : i_a + 1],
                        )
                        i_a += 1

            # ---- final reduction ----
            nc.vector.tensor_reduce(
                red_d[:, :], acc_d[:, :], axis=mybir.AxisListType.X, op=Alu.add
            )
            nc.vector.tensor_reduce(
                red_a[:, :], acc_a[:, :], axis=mybir.AxisListType.X, op=Alu.add
            )
            nc.vector.tensor_copy(m_final[:, 0:1], acc_diag[:, 0:1])
            nc.vector.tensor_add(m_final[:, 1:2], red_d[:, :], red_a[:, :])

            with tc.tile_pool(name="psum_fin", bufs=1, space="PSUM") as pf_pool:
                pf = pf_pool.tile([2, 1], fp32, tag="pf")
                nc.tensor.matmul(
                    pf[:, :], lhsT=m_final[:, :], rhs=ones_sb[:, :],
                    start=True, stop=True,
                )
                nc.vector.tensor_copy(out_sb[:, :], pf[:, :])

            nc.sync.dma_start(d_out[:, :], out_sb[:, :])

    nc.compile()
    return nc


def _get_nc(keep):
    keep = tuple(bool(k) for k in keep)
    if keep not in _BASS_CACHE:
        _BASS_CACHE[keep] = _build_bass(keep)
    return _BASS_CACHE[keep]


def _full_numpy_fallback(f1, f2):
    """Exact reference computation (only used if the screen certificate
    fails, i.e. some pair has d_ij close to or inside the margin)."""
    f1 = f1.astype(np.float32)
    f2 = f2.astype(np.float32)
    n = f1.shape[0]
    sq1 = np.sum(f1 * f1, axis=1)
    sq2 = np.sum(f2 * f2, axis=1)
    total = np.float64(0.0)
    chunk = 512
    for s in range(0, n, chunk):
        e = min(s + chunk, n)
        d2 = sq1[s:e, None] + sq2[None, :] - 2.0 * (f1[s:e] @ f2.T)
        d = np.sqrt(np.maximum(d2, 0.0))
        c = np.maximum(1.0 - d, 0.0)
        for r in range(s, e):
            c[r - s, r] = 0.0
        total += np.float64(np.sum(c * c))
    total += np.float64(np.sum((f1 - f2) ** 2))
    return np.float32(total / (2.0 * n))


def kernel(feature1, feature2):
    global LAST_RESULT
    from concourse.bass_utils import run_bass_kernel_spmd

    f1 = np.ascontiguousarray(np.asarray(feature1, dtype=np.float32))
    f2 = np.ascontiguousarray(np.asarray(feature2, dtype=np.float32))
    assert f1.shape == (N, D) and f2.shape == (N, D)

    bf16 = ml_dtypes.bfloat16
    sq1 = np.sum(f1.astype(np.float64) * f1, axis=1)
    sq2 = np.sum(f2.astype(np.float64) * f2, axis=1)

    # Sort feature2 rows by sq2 so the per-supertile min-sq2 bias is tight.
    perm = np.argsort(sq2, kind="stable")
    f2s = f2[perm]
    sq2s = sq2[perm]
    sq2min = sq2s.reshape(NJH, JW).min(axis=1)  # per j-group minimum
    sq2max = sq2s.reshape(NJH, JW).max(axis=1)

    # fp8 of 2*f2s with exact per-row quantization-error norms; the
    # screen bias absorbs |2dot - z| <= nf1*|dn2| + |dn1|*n2q (C-S).
    fp8 = ml_dtypes.float8_e4m3
    f2q2 = (2.0 * f2s).astype(fp8)                                    # [N, D]
    f2q2f = f2q2.astype(np.float32)
    dn2 = np.linalg.norm(2.0 * f2s.astype(np.float64) - f2q2f, axis=1)
    n2q = np.linalg.norm(f2q2f.astype(np.float64), axis=1)
    maxdn2 = dn2.reshape(NJH, JW).max(axis=1)     # per 1024-col j-group
    maxn2q = n2q.reshape(NJH, JW).max(axis=1)
    f2t2 = np.ascontiguousarray(f2q2.T)                               # [D, N]

    # Shard feature1 by striping the globally-sq1-sorted rows (core c gets
    # sorted rows c::8) so every core's i-tile ti covers the same norm
    # quantile band and the block-skip pattern is core-invariant.
    perm1 = np.argsort(sq1, kind="stable")
    rowids = [perm1[c::NCORES] for c in range(NCORES)]

    # Cauchy-Schwarz block certificate: a span (ti, jh) needs no screening
    # if |norm(f1_i) - norm(f2_j)| >= 1 for all pairs, i.e. the norm
    # intervals are separated by >= 1 (then d2 >= (n1-n2)^2 >= 1 exactly).
    # per-512-group norm intervals (sq2s ascending -> min is first elem)
    g2min = sq2s.reshape(16, 512).min(axis=1)
    g2max = sq2s.reshape(16, 512).max(axis=1)
    keep = []
    for ti in range(R // 128):
        n1lo = np.sqrt(min(sq1[rowids[c][ti * 128]] for c in range(NCORES)))
        n1hi = np.sqrt(max(sq1[rowids[c][(ti + 1) * 128 - 1]]
                           for c in range(NCORES)))
        for jh in range(NJH):
            mode = 0
            for hv in (0, 1):
                g = jh * 2 + hv
                n2lo, n2hi = np.sqrt(g2min[g]), np.sqrt(g2max[g])
                certified = (n2lo - n1hi >= 1.0 + 1e-6) or (
                    n1lo - n2hi >= 1.0 + 1e-6
                )
                if not certified:
                    mode |= 1 << hv
            keep.append(mode)
    kept_idx = [k for k, m in enumerate(keep) if m]

    in_maps = []
    for c in range(NCORES):
        rid = rowids[c]
        f1c_rows = f1[rid]                                            # [R, D]
        sq1c = sq1[rid]
        f1q = f1c_rows.astype(fp8)
        f1qf = f1q.astype(np.float32)
        dn1 = np.linalg.norm(f1c_rows.astype(np.float64) - f1qf, axis=1)
        nf1 = np.sqrt(sq1c)
        s1c = np.empty((128, len(kept_idx)), np.float64)
        for col, k in enumerate(kept_idx):
            ti, jh = k // NJH, k % NJH
            first_half = 0 if (keep[k] & 1) else 1
            rows = slice(ti * 128, (ti + 1) * 128)
            s1c[:, col] = (
                1.0
                - sq1c[rows]
                - g2min[jh * 2 + first_half]
                + (nf1[rows] * maxdn2[jh] + dn1[rows] * maxn2q[jh])
                + 0.05
            )
        # conservative fp32 rounding: bias up (toward firing)
        s1c32 = np.nextafter(s1c.astype(np.float32), np.float32(3.0e38))
        in_maps.append(
            {
                "f2t2": f2t2,
                "f1t": np.ascontiguousarray(f1q.T),
                "s1c": np.ascontiguousarray(s1c32),
                "diff": np.ascontiguousarray(
                    (f1c_rows.reshape(128, R) - f2[rid].reshape(128, R))
                    .astype(bf16)
                ),
            }
        )

    nc = _get_nc(keep)
    res = run_bass_kernel_spmd(
        nc,
        in_maps,
        core_ids=list(range(NCORES)),
        trace=TRACE,
        **TRACE_KWARGS,
    )
    LAST_RESULT = res

    diag_total = np.float64(0.0)
    screen_total = np.float64(0.0)
    for r in res.results:
        out = r["out"]
        diag_total += np.float64(out[0, 0])
        screen_total += np.float64(out[1, 0])

    if screen_total != 0.0:
        return _full_numpy_fallback(f1, f2)

    return np.float32(diag_total / (2.0 * N))

